# revision 20
# baseline (speedup 1.0000x reference)
"""Trainium2 Bass kernel for the Covid compartment forecast model.

Math (per posterior sample s, forecast day t in [0, T)):
    growth[t,s] = r_t[t]**(1/T_serial[s]) * delta[s]
    A[t,s]      = A[t-1,s] * growth[t,s],  A[-1] = warmup[-1]
    M[t,s]      = sum_j A_full[J-1-j+t, s] * rho[s] * pi[j, s]

The sequential scan is replaced by its closed form
    A[t,s] = exp(b[s] + (t+1)*ln(delta[s]) + invT[s] * L[t]),
    L[t] = cumsum(ln r_t)[t],  b[s] = ln(warmup[-1, s])
so each 128-sample tile is fully parallel: one DVE scalar_tensor_tensor
builds the exponent, one ACT Exp (per-partition scale/bias) emits A in
bf16. The 32-tap FIR runs in bf16 split across engines by measured cost:
  PE   : diag(q_j) matmuls accumulated in PSUM        (~432ns/tap)
  ACT  : scaled copies B_j = A_shift * q_j            (~1131ns/tap)
  DVE  : tensor_scalar product + tensor_tensor adds   (~928ns/tap)
ACT products are pair-summed on DVE, then everything funnels through
identity matmuls into the same PSUM accumulation. M leaves as bf16 in
[sample, day] layout; the host does the final transpose/concat.

Sharding: posterior-sample dimension S across 8 cores (data parallel).
"""

import numpy as np

import concourse.bacc as bacc
import concourse.bass as bass
import concourse.mybir as mybir
import concourse.tile as tile
from concourse.bass_utils import run_bass_kernel_spmd

F32 = mybir.dt.float32
BF16 = mybir.dt.bfloat16
I32 = mybir.dt.int32
AF = mybir.ActivationFunctionType
OP = mybir.AluOpType

T = 1024
J = 32
S_TOTAL = 50000
NCORES = 8
P = 128
S_CORE = S_TOTAL // NCORES           # 6250
NTILES = (S_CORE + P - 1) // P       # 49
S_PAD = NTILES * P                   # 6272

# Tap -> engine. Offsets are 31-j; DVE taps take odd j (even offsets,
# needed for the 4x tensor_scalar perf mode on bf16). PE diag weight
# matrices are prebuilt on the host and DMA'd (DMA engines are idle).
PE_TAPS = tuple(range(0, 20))
ACT_TAPS = (20, 22, 24, 26, 28, 30)
DVE_TAPS = (21, 23, 25, 27, 29, 31)


def build():
    taps = sorted(PE_TAPS + ACT_TAPS + DVE_TAPS)
    assert taps == list(range(J))

    nc = bacc.Bacc("TRN2", target_bir_lowering=False, debug=False,
                   num_devices=NCORES)
    it1 = nc.dram_tensor("iota1", [1, T], F32, kind="ExternalInput").ap()
    lc = nc.dram_tensor("lcum", [1, T], F32, kind="ExternalInput").ap()
    q = nc.dram_tensor("q", [S_PAD, J], F32, kind="ExternalInput").ap()
    wu = nc.dram_tensor("wu", [S_PAD, J], BF16, kind="ExternalInput").ap()
    sc = nc.dram_tensor("scal", [S_PAD, 4], F32, kind="ExternalInput").ap()
    qd = nc.dram_tensor("qdiag", [S_PAD, len(PE_TAPS) * P], BF16,
                        kind="ExternalInput").ap()
    m = nc.dram_tensor("m_out", [S_PAD, T], BF16, kind="ExternalOutput").ap()

    def bcast(a):
        return bass.AP(tensor=a.tensor, offset=a.offset, ap=[[0, P], [1, T]])

    with tile.TileContext(nc) as tc:
        with (
            tc.tile_pool(name="singles", bufs=1) as singles,
            tc.tile_pool(name="loads", bufs=5) as loads,
            tc.tile_pool(name="argp", bufs=4) as argp,
            tc.tile_pool(name="apool", bufs=4) as apool,
            tc.tile_pool(name="bpool", bufs=8) as bpool,
            tc.tile_pool(name="cpool", bufs=4) as cpool,
            tc.tile_pool(name="mdp", bufs=3) as mdp,
            tc.tile_pool(name="msb", bufs=3) as msb,
            tc.tile_pool(name="mpsum", bufs=3, space="PSUM") as mpsum,
            tc.tile_pool(name="wpsum", bufs=1, space="PSUM") as wpsum,
        ):
            # ---- one-time setup ----
            iota_t = singles.tile([P, P], I32)
            nc.gpsimd.iota(iota_t, pattern=[[1, P]], base=0,
                           channel_multiplier=-1)
            ident = singles.tile([P, P], BF16)
            nc.vector.tensor_scalar(out=ident, in0=iota_t, scalar1=0,
                                    scalar2=None, op0=OP.is_equal)
            L_bc = singles.tile([P, T], F32)
            nc.sync.dma_start(out=L_bc, in_=bcast(lc))
            # it_bc = 1..T on every partition, generated on-chip (cheaper
            # than a 512KB broadcast DMA on the tile-0 critical path)
            it_i = singles.tile([P, T], I32)
            nc.gpsimd.iota(it_i, pattern=[[1, T]], base=1,
                           channel_multiplier=0)
            it_bc = singles.tile([P, T], F32)
            nc.vector.tensor_scalar(out=it_bc, in0=it_i, scalar1=0,
                                    scalar2=None, op0=OP.add)
            # keep the PE busy while tile-0 inputs stream in, so HAM is
            # already unthrottled (2.4 GHz) when the real matmuls start
            warm = wpsum.tile([P, P], F32)
            for _ in range(110):
                nc.tensor.matmul(warm, ident, ident, start=True, stop=True)

            # ---- per sample-tile ----
            pending = None
            for i in range(NTILES):
                s0 = i * P

                qt = loads.tile([P, J], F32, tag="q")
                nc.sync.dma_start(out=qt, in_=q[s0:s0 + P, :])
                sct = loads.tile([P, 4], F32, tag="sc")
                nc.sync.dma_start(out=sct, in_=sc[s0:s0 + P, :])
                dgs = loads.tile([P, len(PE_TAPS) * P], BF16, tag="dgs")
                nc.sync.dma_start(out=dgs, in_=qd[s0:s0 + P, :])

                # exponent: arg[s,t] = (t+1)*w[s] + L[t]   (w = lnd*T_serial)
                argt = argp.tile([P, T], F32)
                nc.vector.scalar_tensor_tensor(
                    out=argt, in0=it_bc, scalar=sct[:, 0:1], in1=L_bc,
                    op0=OP.mult, op1=OP.add)

                # A_full[:, 0:J] = warmup (DMA, bf16), A_full[:, J:] = exp(...)
                A = apool.tile([P, J + T], BF16)
                nc.sync.dma_start(out=A[:, 0:J], in_=wu[s0:s0 + P, :])
                nc.scalar.activation(out=A[:, J:J + T], in_=argt, func=AF.Exp,
                                     bias=sct[:, 2:3], scale=sct[:, 1:2])

                # ---- FIR: M[t] = sum_j q[j] * A_full[31-j+t] ----
                Mp = mpsum.tile([P, T], F32)
                started = [False, False]

                def pe_acc(lhsT, rhs_base, stop=False):
                    for c in range(2):
                        lo = c * 512
                        nc.tensor.matmul(
                            Mp[:, lo:lo + 512], lhsT,
                            rhs_base[:, lo:lo + 512],
                            start=not started[c], stop=stop)
                        started[c] = True

                for k, j in enumerate(PE_TAPS):
                    pe_acc(dgs[:, k * P:(k + 1) * P],
                           A[:, J - 1 - j:J - 1 - j + T])

                # ACT taps -> pairwise sums on DVE -> PE merge.
                # Odd tiles shift tap 21 DVE->ACT so the time-averaged
                # engine load equalizes (taps are integer-granular).
                act_taps = ACT_TAPS if i % 2 == 0 else ACT_TAPS + (21,)
                dve_taps = DVE_TAPS if i % 2 == 0 else (23, 25, 27, 29, 31)
                Bs = []
                for j in act_taps:
                    B = bpool.tile([P, T], BF16, tag="b")
                    nc.scalar.activation(out=B,
                                         in_=A[:, J - 1 - j:J - 1 - j + T],
                                         func=AF.Copy, scale=qt[:, j:j + 1])
                    Bs.append(B)
                Cs = []
                for k in range(0, 6, 2):
                    C = cpool.tile([P, T], BF16, tag=f"c{k}")
                    nc.vector.tensor_tensor(out=C, in0=Bs[k], in1=Bs[k + 1],
                                            op=OP.add)
                    Cs.append(C)
                CC = cpool.tile([P, T], BF16, tag="cc")
                nc.vector.tensor_tensor(out=CC, in0=Cs[0], in1=Cs[1],
                                        op=OP.add)
                pe_acc(ident, CC)
                lastC = Cs[2]

                # deferred PSUM->SBUF copy of the PREVIOUS tile: keeps
                # this tile's Exp/products ahead of it in ACT's in-order
                # queue (the copy waits on the PE stop semaphore).
                if pending is not None:
                    Mp_prev, s0_prev = pending
                    M_sb = msb.tile([P, T], BF16)
                    nc.scalar.activation(out=M_sb, in_=Mp_prev, func=AF.Copy)
                    nc.sync.dma_start(out=m[s0_prev:s0_prev + P, :], in_=M_sb)
                    pending = None

                # DVE taps: TS products + balanced tree of TT adds;
                # odd-tile leftover ACT product joins the tree
                prods = list(Bs[6:])
                for k, j in enumerate(dve_taps):
                    Bt = mdp.tile([P, T], BF16, tag=f"bt{k}")
                    nc.vector.tensor_scalar(out=Bt,
                                            in0=A[:, J - 1 - j:J - 1 - j + T],
                                            scalar1=qt[:, j:j + 1],
                                            scalar2=None, op0=OP.mult)
                    prods.append(Bt)
                lvl = 0
                while len(prods) > 1:
                    nxt = []
                    for k in range(0, len(prods) - 1, 2):
                        Sm = mdp.tile([P, T], BF16, tag=f"s{lvl}{k}")
                        nc.vector.tensor_tensor(out=Sm, in0=prods[k],
                                                in1=prods[k + 1], op=OP.add)
                        nxt.append(Sm)
                    if len(prods) % 2:
                        nxt.append(prods[-1])
                    prods = nxt
                    lvl += 1
                Md = prods[0]
                # fold the last ACT pair into Md on DVE: one less PE merge
                Mf = mdp.tile([P, T], BF16, tag="mf")
                nc.vector.tensor_tensor(out=Mf, in0=Md, in1=lastC, op=OP.add)
                pe_acc(ident, Mf, stop=True)

                pending = (Mp, s0)

            # flush the last tile's output, chunked so the first DMA
            # overlaps the second copy (this copy is on the critical tail)
            Mp_prev, s0_prev = pending
            M_sb = msb.tile([P, T], BF16)
            nc.scalar.activation(out=M_sb[:, 0:512], in_=Mp_prev[:, 0:512],
                                 func=AF.Copy)
            nc.sync.dma_start(out=m[s0_prev:s0_prev + P, 0:512],
                              in_=M_sb[:, 0:512])
            nc.scalar.activation(out=M_sb[:, 512:1024],
                                 in_=Mp_prev[:, 512:1024], func=AF.Copy)
            nc.sync.dma_start(out=m[s0_prev:s0_prev + P, 512:1024],
                              in_=M_sb[:, 512:1024])

    nc.compile()
    return nc


_NC_CACHE = {}


def _get_nc():
    key = (S_PAD, PE_TAPS, ACT_TAPS, DVE_TAPS)
    if key not in _NC_CACHE:
        _NC_CACHE[key] = build()
    return _NC_CACHE[key]


def _prep_inputs(r_t, warmup_A, delta, T_serial, rho_M, pi_M):
    """Host-side parameter prep + per-core sharding along S."""
    r_t = np.asarray(r_t, dtype=np.float32)
    warmup_A = np.asarray(warmup_A, dtype=np.float32)
    delta = np.asarray(delta, dtype=np.float32)
    T_serial = np.asarray(T_serial, dtype=np.float32)
    rho_M = np.asarray(rho_M, dtype=np.float32)
    pi_M = np.asarray(pi_M, dtype=np.float32)

    iota1 = np.arange(1, T + 1, dtype=np.float32).reshape(1, T)
    lcum = np.cumsum(np.log(r_t), dtype=np.float32).reshape(1, T)
    lnd = np.log(delta)
    q_full = (rho_M[None, :] * pi_M).T.astype(np.float32)       # [S, J]
    wu_full = warmup_A.T.astype(np.float32)                      # [S, J]
    w_full = lnd * T_serial
    invT_full = (1.0 / T_serial).astype(np.float32)
    b_full = np.log(warmup_A[-1]).astype(np.float32)

    import ml_dtypes

    q16 = np.right_shift(q_full.view(np.uint32) + 0x8000, 16).astype(np.uint16)

    pad = S_PAD - S_CORE
    in_maps = []
    for c in range(NCORES):
        lo, hi = c * S_CORE, (c + 1) * S_CORE

        def pad2(a, fill):
            return np.pad(a[lo:hi], ((0, pad), (0, 0)), constant_values=fill)

        scal = np.stack([w_full[lo:hi], invT_full[lo:hi], b_full[lo:hi],
                         np.zeros(S_CORE, np.float32)], axis=1)
        # padded lanes: w=-1, invT=1, b=0 -> A decays, q=0 -> M=0
        scal = np.pad(scal, ((0, pad), (0, 0)), constant_values=0.0)
        scal[S_CORE:, 0] = -1.0
        scal[S_CORE:, 1] = 1.0

        qd = np.zeros((S_PAD, len(PE_TAPS) * P), dtype=np.uint16)
        idx = np.arange(S_CORE)
        for k, j in enumerate(PE_TAPS):
            qd[idx, k * P + (idx % P)] = q16[lo:hi, j]
        in_maps.append({
            "iota1": iota1,
            "lcum": lcum,
            "q": pad2(q_full, 0.0),
            "wu": np.right_shift(
                pad2(wu_full, 1.0).view(np.uint32) + 0x8000, 16
            ).astype(np.uint16).view(ml_dtypes.bfloat16),
            "scal": np.ascontiguousarray(scal),
            "qdiag": qd.view(ml_dtypes.bfloat16),
        })
    return in_maps


def _bf16_to_f32(a):
    a = np.asarray(a)
    if a.dtype == np.float32:
        return a
    u = a.view(np.uint16).astype(np.uint32) << 16
    return u.view(np.float32)


def run(inputs, trace=False, **kwargs):
    """Run on 8 cores; returns (M [T, S_TOTAL] float32, BassKernelResults)."""
    nc = _get_nc()
    in_maps = _prep_inputs(**inputs)
    res = run_bass_kernel_spmd(nc, in_maps, core_ids=list(range(NCORES)),
                               trace=trace, **kwargs)
    cols = []
    for c in range(NCORES):
        mc = _bf16_to_f32(res.results[c]["m_out"])[:S_CORE]   # [S_CORE, T]
        cols.append(mc.T)
    M = np.concatenate(cols, axis=1)
    return np.ascontiguousarray(M, dtype=np.float32), res


def kernel(**inputs):
    M, _ = run(inputs)
    return M


# revision 21
# speedup vs baseline: 1.0006x; 1.0006x over previous
"""Trainium2 Bass kernel for the Covid compartment forecast model.

Math (per posterior sample s, forecast day t in [0, T)):
    growth[t,s] = r_t[t]**(1/T_serial[s]) * delta[s]
    A[t,s]      = A[t-1,s] * growth[t,s],  A[-1] = warmup[-1]
    M[t,s]      = sum_j A_full[J-1-j+t, s] * rho[s] * pi[j, s]

The sequential scan is replaced by its closed form
    A[t,s] = exp(b[s] + (t+1)*ln(delta[s]) + invT[s] * L[t]),
    L[t] = cumsum(ln r_t)[t],  b[s] = ln(warmup[-1, s])
so each 128-sample tile is fully parallel: one DVE scalar_tensor_tensor
builds the exponent, one ACT Exp (per-partition scale/bias) emits A in
bf16. The 32-tap FIR runs in bf16 split across engines by measured cost:
  PE   : diag(q_j) matmuls accumulated in PSUM        (~432ns/tap)
  ACT  : scaled copies B_j = A_shift * q_j            (~1131ns/tap)
  DVE  : tensor_scalar product + tensor_tensor adds   (~928ns/tap)
ACT products are pair-summed on DVE, then everything funnels through
identity matmuls into the same PSUM accumulation. M leaves as bf16 in
[sample, day] layout; the host does the final transpose/concat.

Sharding: posterior-sample dimension S across 8 cores (data parallel).
"""

import numpy as np

import concourse.bacc as bacc
import concourse.bass as bass
import concourse.mybir as mybir
import concourse.tile as tile
from concourse.bass_utils import run_bass_kernel_spmd

F32 = mybir.dt.float32
BF16 = mybir.dt.bfloat16
I32 = mybir.dt.int32
AF = mybir.ActivationFunctionType
OP = mybir.AluOpType

T = 1024
J = 32
S_TOTAL = 50000
NCORES = 8
P = 128
S_CORE = S_TOTAL // NCORES           # 6250
NTILES = (S_CORE + P - 1) // P       # 49
S_PAD = NTILES * P                   # 6272

# Tap -> engine. Offsets are 31-j; DVE taps take odd j (even offsets,
# needed for the 4x tensor_scalar perf mode on bf16). PE diag weight
# matrices are prebuilt on the host and DMA'd (DMA engines are idle).
PE_TAPS = tuple(range(0, 20))
ACT_TAPS = (20, 22, 24, 26, 28, 30)
DVE_TAPS = (21, 23, 25, 27, 29, 31)


def build():
    taps = sorted(PE_TAPS + ACT_TAPS + DVE_TAPS)
    assert taps == list(range(J))

    nc = bacc.Bacc("TRN2", target_bir_lowering=False, debug=False,
                   num_devices=NCORES)
    it1 = nc.dram_tensor("iota1", [1, T], F32, kind="ExternalInput").ap()
    lc = nc.dram_tensor("lcum", [1, T], F32, kind="ExternalInput").ap()
    q = nc.dram_tensor("q", [S_PAD, J], F32, kind="ExternalInput").ap()
    wu = nc.dram_tensor("wu", [S_PAD, J], BF16, kind="ExternalInput").ap()
    sc = nc.dram_tensor("scal", [S_PAD, 4], F32, kind="ExternalInput").ap()
    qd = nc.dram_tensor("qdiag", [S_PAD, len(PE_TAPS) * P], BF16,
                        kind="ExternalInput").ap()
    m = nc.dram_tensor("m_out", [S_PAD, T], BF16, kind="ExternalOutput").ap()

    def bcast(a):
        return bass.AP(tensor=a.tensor, offset=a.offset, ap=[[0, P], [1, T]])

    with tile.TileContext(nc) as tc:
        with (
            tc.tile_pool(name="singles", bufs=1) as singles,
            tc.tile_pool(name="loads", bufs=5) as loads,
            tc.tile_pool(name="argp", bufs=4) as argp,
            tc.tile_pool(name="apool", bufs=4) as apool,
            tc.tile_pool(name="bpool", bufs=6) as bpool,
            tc.tile_pool(name="cpool", bufs=4) as cpool,
            tc.tile_pool(name="mdp", bufs=2) as mdp,
            tc.tile_pool(name="msb", bufs=4) as msb,
            tc.tile_pool(name="mpsum", bufs=3, space="PSUM") as mpsum,
            tc.tile_pool(name="wpsum", bufs=1, space="PSUM") as wpsum,
        ):
            # ---- one-time setup ----
            iota_t = singles.tile([P, P], I32)
            nc.gpsimd.iota(iota_t, pattern=[[1, P]], base=0,
                           channel_multiplier=-1)
            ident = singles.tile([P, P], BF16)
            nc.vector.tensor_scalar(out=ident, in0=iota_t, scalar1=0,
                                    scalar2=None, op0=OP.is_equal)
            L_bc = singles.tile([P, T], F32)
            nc.sync.dma_start(out=L_bc, in_=bcast(lc))
            # it_bc = 1..T on every partition, generated on-chip (cheaper
            # than a 512KB broadcast DMA on the tile-0 critical path)
            it_i = singles.tile([P, T], I32)
            nc.gpsimd.iota(it_i, pattern=[[1, T]], base=1,
                           channel_multiplier=0)
            it_bc = singles.tile([P, T], F32)
            nc.vector.tensor_scalar(out=it_bc, in0=it_i, scalar1=0,
                                    scalar2=None, op0=OP.add)
            # keep the PE busy while tile-0 inputs stream in, so HAM is
            # already unthrottled (2.4 GHz) when the real matmuls start
            warm = wpsum.tile([P, P], F32)
            for _ in range(110):
                nc.tensor.matmul(warm, ident, ident, start=True, stop=True)

            # ---- per sample-tile ----
            pending = None
            for i in range(NTILES):
                s0 = i * P

                qt = loads.tile([P, J], F32, tag="q")
                nc.sync.dma_start(out=qt, in_=q[s0:s0 + P, :])
                sct = loads.tile([P, 4], F32, tag="sc")
                nc.sync.dma_start(out=sct, in_=sc[s0:s0 + P, :])
                dgs = loads.tile([P, len(PE_TAPS) * P], BF16, tag="dgs")
                nc.sync.dma_start(out=dgs, in_=qd[s0:s0 + P, :])

                # exponent: arg[s,t] = (t+1)*w[s] + L[t]   (w = lnd*T_serial)
                argt = argp.tile([P, T], F32)
                nc.vector.scalar_tensor_tensor(
                    out=argt, in0=it_bc, scalar=sct[:, 0:1], in1=L_bc,
                    op0=OP.mult, op1=OP.add)

                # A_full[:, 0:J] = warmup (DMA, bf16), A_full[:, J:] = exp(...)
                A = apool.tile([P, J + T], BF16)
                nc.sync.dma_start(out=A[:, 0:J], in_=wu[s0:s0 + P, :])
                nc.scalar.activation(out=A[:, J:J + T], in_=argt, func=AF.Exp,
                                     bias=sct[:, 2:3], scale=sct[:, 1:2])

                # ---- FIR: M[t] = sum_j q[j] * A_full[31-j+t] ----
                Mp = mpsum.tile([P, T], F32)
                started = [False, False]

                def pe_acc(lhsT, rhs_base, stop=False):
                    for c in range(2):
                        lo = c * 512
                        nc.tensor.matmul(
                            Mp[:, lo:lo + 512], lhsT,
                            rhs_base[:, lo:lo + 512],
                            start=not started[c], stop=stop)
                        started[c] = True

                for k, j in enumerate(PE_TAPS):
                    pe_acc(dgs[:, k * P:(k + 1) * P],
                           A[:, J - 1 - j:J - 1 - j + T])

                # ACT taps -> pairwise sums on DVE -> PE merge
                Bs = []
                for j in ACT_TAPS:
                    B = bpool.tile([P, T], BF16, tag="b")
                    nc.scalar.activation(out=B,
                                         in_=A[:, J - 1 - j:J - 1 - j + T],
                                         func=AF.Copy, scale=qt[:, j:j + 1])
                    Bs.append(B)
                Cs = []
                for k in range(0, len(Bs), 2):
                    C = cpool.tile([P, T], BF16, tag=f"c{k}")
                    nc.vector.tensor_tensor(out=C, in0=Bs[k], in1=Bs[k + 1],
                                            op=OP.add)
                    Cs.append(C)
                CC = cpool.tile([P, T], BF16, tag="cc")
                nc.vector.tensor_tensor(out=CC, in0=Cs[0], in1=Cs[1],
                                        op=OP.add)
                pe_acc(ident, CC)
                lastC = Cs[2]

                # deferred PSUM->SBUF copy of the PREVIOUS tile: keeps
                # this tile's Exp/products ahead of it in ACT's in-order
                # queue (the copy waits on the PE stop semaphore).
                if pending is not None:
                    Mp_prev, s0_prev = pending
                    M_sb = msb.tile([P, T], BF16)
                    nc.scalar.activation(out=M_sb, in_=Mp_prev, func=AF.Copy)
                    nc.sync.dma_start(out=m[s0_prev:s0_prev + P, :], in_=M_sb)
                    pending = None

                # DVE taps: TS products + balanced tree of TT adds
                prods = []
                for k, j in enumerate(DVE_TAPS):
                    Bt = mdp.tile([P, T], BF16, tag=f"bt{k}")
                    nc.vector.tensor_scalar(out=Bt,
                                            in0=A[:, J - 1 - j:J - 1 - j + T],
                                            scalar1=qt[:, j:j + 1],
                                            scalar2=None, op0=OP.mult)
                    prods.append(Bt)
                lvl = 0
                while len(prods) > 1:
                    nxt = []
                    for k in range(0, len(prods) - 1, 2):
                        Sm = mdp.tile([P, T], BF16, tag=f"s{lvl}{k}")
                        nc.vector.tensor_tensor(out=Sm, in0=prods[k],
                                                in1=prods[k + 1], op=OP.add)
                        nxt.append(Sm)
                    if len(prods) % 2:
                        nxt.append(prods[-1])
                    prods = nxt
                    lvl += 1
                Md = prods[0]
                # fold the last ACT pair into Md on DVE: one less PE merge
                Mf = mdp.tile([P, T], BF16, tag="mf")
                nc.vector.tensor_tensor(out=Mf, in0=Md, in1=lastC, op=OP.add)
                pe_acc(ident, Mf, stop=True)

                pending = (Mp, s0)

            # flush the last tile's output, chunked so the first DMA
            # overlaps the second copy (this copy is on the critical tail)
            Mp_prev, s0_prev = pending
            M_sb = msb.tile([P, T], BF16)
            nc.scalar.activation(out=M_sb[:, 0:512], in_=Mp_prev[:, 0:512],
                                 func=AF.Copy)
            nc.sync.dma_start(out=m[s0_prev:s0_prev + P, 0:512],
                              in_=M_sb[:, 0:512])
            nc.scalar.activation(out=M_sb[:, 512:1024],
                                 in_=Mp_prev[:, 512:1024], func=AF.Copy)
            nc.sync.dma_start(out=m[s0_prev:s0_prev + P, 512:1024],
                              in_=M_sb[:, 512:1024])

    nc.compile()
    return nc


_NC_CACHE = {}


def _get_nc():
    key = (S_PAD, PE_TAPS, ACT_TAPS, DVE_TAPS)
    if key not in _NC_CACHE:
        _NC_CACHE[key] = build()
    return _NC_CACHE[key]


def _prep_inputs(r_t, warmup_A, delta, T_serial, rho_M, pi_M):
    """Host-side parameter prep + per-core sharding along S."""
    r_t = np.asarray(r_t, dtype=np.float32)
    warmup_A = np.asarray(warmup_A, dtype=np.float32)
    delta = np.asarray(delta, dtype=np.float32)
    T_serial = np.asarray(T_serial, dtype=np.float32)
    rho_M = np.asarray(rho_M, dtype=np.float32)
    pi_M = np.asarray(pi_M, dtype=np.float32)

    iota1 = np.arange(1, T + 1, dtype=np.float32).reshape(1, T)
    lcum = np.cumsum(np.log(r_t), dtype=np.float32).reshape(1, T)
    lnd = np.log(delta)
    q_full = (rho_M[None, :] * pi_M).T.astype(np.float32)       # [S, J]
    wu_full = warmup_A.T.astype(np.float32)                      # [S, J]
    w_full = lnd * T_serial
    invT_full = (1.0 / T_serial).astype(np.float32)
    b_full = np.log(warmup_A[-1]).astype(np.float32)

    import ml_dtypes

    q16 = np.right_shift(q_full.view(np.uint32) + 0x8000, 16).astype(np.uint16)

    pad = S_PAD - S_CORE
    in_maps = []
    for c in range(NCORES):
        lo, hi = c * S_CORE, (c + 1) * S_CORE

        def pad2(a, fill):
            return np.pad(a[lo:hi], ((0, pad), (0, 0)), constant_values=fill)

        scal = np.stack([w_full[lo:hi], invT_full[lo:hi], b_full[lo:hi],
                         np.zeros(S_CORE, np.float32)], axis=1)
        # padded lanes: w=-1, invT=1, b=0 -> A decays, q=0 -> M=0
        scal = np.pad(scal, ((0, pad), (0, 0)), constant_values=0.0)
        scal[S_CORE:, 0] = -1.0
        scal[S_CORE:, 1] = 1.0

        qd = np.zeros((S_PAD, len(PE_TAPS) * P), dtype=np.uint16)
        idx = np.arange(S_CORE)
        for k, j in enumerate(PE_TAPS):
            qd[idx, k * P + (idx % P)] = q16[lo:hi, j]
        in_maps.append({
            "iota1": iota1,
            "lcum": lcum,
            "q": pad2(q_full, 0.0),
            "wu": np.right_shift(
                pad2(wu_full, 1.0).view(np.uint32) + 0x8000, 16
            ).astype(np.uint16).view(ml_dtypes.bfloat16),
            "scal": np.ascontiguousarray(scal),
            "qdiag": qd.view(ml_dtypes.bfloat16),
        })
    return in_maps


def _bf16_to_f32(a):
    a = np.asarray(a)
    if a.dtype == np.float32:
        return a
    u = a.view(np.uint16).astype(np.uint32) << 16
    return u.view(np.float32)


def run(inputs, trace=False, **kwargs):
    """Run on 8 cores; returns (M [T, S_TOTAL] float32, BassKernelResults)."""
    nc = _get_nc()
    in_maps = _prep_inputs(**inputs)
    res = run_bass_kernel_spmd(nc, in_maps, core_ids=list(range(NCORES)),
                               trace=trace, **kwargs)
    cols = []
    for c in range(NCORES):
        mc = _bf16_to_f32(res.results[c]["m_out"])[:S_CORE]   # [S_CORE, T]
        cols.append(mc.T)
    M = np.concatenate(cols, axis=1)
    return np.ascontiguousarray(M, dtype=np.float32), res


def kernel(**inputs):
    M, _ = run(inputs)
    return M
